# revision 36
# baseline (speedup 1.0000x reference)
"""Causal multi-head self-attention on 8 TRN2 NeuronCores.

Sharding: tensor-parallel over heads. 16 heads / 8 cores = 2 heads per core.
Each core computes q/k/v projections for its 2 heads (feature-major via
fp32r matmuls), block-causal attention (scores kept k-major so softmax sums
come from a fused ones-column in the attn@v matmul and no transposes are
needed), and a partial output projection against its 128-column slice of
W_O. The host sums the 8 partial outputs.

Layouts on core c (heads 2c, 2c+1 = "A", "B"):
  qT/kT  [128, 2048]  feature-major; rows 0:64 head A dk, 64:128 head B
  vtok   [128, 16, 192] token-major v (PE-transposed): cols 0:64 vA, col 64
         a shared ones column, cols 128:192 vB. Head A's ctx lhsT slice is
         cols 0:128 (ctxA on partitions 0:64, sumA on 64); head B's slice is
         cols 64:192, so the same ones column becomes its col 0 (sumB on
         partition 0) and vB lands on partitions 64:128 — no partition-shift
         DMA is needed to assemble ctxn.
  scoresT[128 k-tok, <=512 q-tok] per (q-tile, k-tile); exp'd on ScalarE
  ctxAB  [128, 2*512] PSUM; A half rows 0:64 + sum row 64, B half sum row 0
         + rows 64:128
  out    partial [1024, 8192] feature-major; host sums over cores + transposes

Pipelining: the whole kernel is one deferred-work machine. Attention of
batch b interleaves (as pop-closures in its k-tile loop) the q/k/v
projections + v-transposes of batch b+1, plus the softmax-normalization
(finish_rb) and output-projection (finish_oproj) of the previous q-tile.
This keeps the PE stream dense (pstate stays at max clock) and gives the
scalar engine's exp stream (~1us per k-iter, the attention pacer) slack.

Softmax denominators: sumA sits on partition 64, sumB on partition 0 of the
two ctx halves; both are same-partition scalar copies into s2, whose rows
1:63 hold persistent zeros, and a K=65 indicator matmul broadcasts them to
[128, QTILE]. reciprocal_approx_fast (~18 bits) + one DVE multiply
normalize ctx.

Toolchain constraints honored here: col-offset tile_position is illegal for
4-byte matmul dtypes; fp32r consumers need fp32r-typed producers; x is
transposed on the host so every DMA has a contiguous innermost run (>=2KB);
fp32r matmuls with moving dim >= 256 run at full PE rate.
"""

import numpy as np
from contextlib import ExitStack

import concourse.bass as bass
import concourse.tile as tile
from concourse import bacc, mybir
from concourse.bass_utils import run_bass_kernel_spmd

F32 = mybir.dt.float32
F32R = mybir.dt.float32r
BF16 = mybir.dt.bfloat16

B, S, D, H = 4, 2048, 1024, 16
DK = D // H  # 64
NCORES = 8
T = B * S  # 8192 tokens
KT = D // 128  # 8 contraction tiles for projections
QTILE = 512  # q-tile width (tokens)
KTILE = 128  # k-tile width (tokens)
NQT = S // QTILE  # 4 q-tiles per batch
NKT = S // KTILE  # 16 k-tiles per batch
LAG = 2  # ctx matmuls trail scores by this many k-tiles
EXP_FUNC = mybir.ActivationFunctionType.Exp
INV_SQRT_DK = 1.0 / np.sqrt(DK)


def build_nc():
    nc = bacc.Bacc("TRN2", target_bir_lowering=False, debug=False)

    xT = nc.dram_tensor("xT", [D, T], F32, kind="ExternalInput").ap()
    wq = nc.dram_tensor("wq", [D, 128], F32, kind="ExternalInput").ap()
    wk = nc.dram_tensor("wk", [D, 128], F32, kind="ExternalInput").ap()
    wv = nc.dram_tensor("wv", [D, 128], F32, kind="ExternalInput").ap()
    wo = nc.dram_tensor("wo", [128, D], F32, kind="ExternalInput").ap()
    tri = nc.dram_tensor("tri", [128, 128], BF16, kind="ExternalInput").ap()
    ind = nc.dram_tensor("ind", [65, 128], F32, kind="ExternalInput").ap()
    ident = nc.dram_tensor("ident", [128, 128], BF16, kind="ExternalInput").ap()
    outT = nc.dram_tensor("outT", [D, T], F32, kind="ExternalOutput").ap()

    xT_r = xT.rearrange("(kt p) t -> p kt t", p=128)
    outT_r = outT.rearrange("(jt p) t -> p jt t", p=128)

    with ExitStack() as ctx:
        tc = ctx.enter_context(tile.TileContext(nc))
        consts = ctx.enter_context(tc.tile_pool(name="consts", bufs=1))
        xt_pool = ctx.enter_context(tc.tile_pool(name="xt_pool", bufs=3))
        batch_pool = ctx.enter_context(tc.tile_pool(name="batch_pool", bufs=2))
        vtmp_pool = ctx.enter_context(tc.tile_pool(name="vtmp_pool", bufs=3))
        exp_pool = ctx.enter_context(tc.tile_pool(name="exp_pool", bufs=4))
        ctxn_pool = ctx.enter_context(tc.tile_pool(name="ctxn_pool", bufs=2))
        oall_pool = ctx.enter_context(tc.tile_pool(name="oall_pool", bufs=1))
        small_pool = ctx.enter_context(tc.tile_pool(name="small_pool", bufs=2))
        ps = ctx.enter_context(tc.tile_pool(name="ps", bufs=1, space="PSUM"))

        # --- constants / weights (persistent) ---
        # Ordered so the head of the kernel only waits for wq + the first
        # x chunk: wo (first needed ~40us in) and small consts go later.
        wq_sb = consts.tile([128, KT, 128], F32R)
        wq_src = wq.rearrange("(kt p) m -> p kt m", p=128).bitcast(F32R)
        nc.sync.dma_start(out=wq_sb[:, 0 : KT // 2, :], in_=wq_src[:, 0 : KT // 2, :])
        nc.sync.dma_start(out=wq_sb[:, KT // 2 :, :], in_=wq_src[:, KT // 2 :, :])
        # first x tile, split per-kt and issued directly after wq so the
        # first projection matmul starts after ~768KB instead of ~4MB
        xt00 = xt_pool.tile([128, KT, QTILE], F32R, name="xt", tag="xt")
        for kt in range(KT):
            nc.sync.dma_start(
                out=xt00[:, kt, :], in_=xT_r[:, kt, 0:QTILE].bitcast(F32R)
            )
        wk_sb = consts.tile([128, KT, 128], F32R)
        nc.sync.dma_start(
            out=wk_sb, in_=wk.rearrange("(kt p) m -> p kt m", p=128).bitcast(F32R)
        )
        wv_sb = consts.tile([128, KT, 128], F32R)
        nc.sync.dma_start(
            out=wv_sb, in_=wv.rearrange("(kt p) m -> p kt m", p=128).bitcast(F32R)
        )
        tri_sb = consts.tile([128, 128], BF16)
        nc.sync.dma_start(out=tri_sb, in_=tri)
        ind_sb = consts.tile([65, 128], F32R)
        nc.sync.dma_start(out=ind_sb, in_=ind.bitcast(F32R))
        ident_sb = consts.tile([128, 128], BF16)
        nc.sync.dma_start(out=ident_sb, in_=ident)
        wo_sb = consts.tile([128, KT, 128], F32R)
        nc.sync.dma_start(
            out=wo_sb, in_=wo.rearrange("p (jt m) -> p jt m", jt=KT).bitcast(F32R)
        )

        finishq = []  # [finish_rb(qi), finish_oproj(qi)] of the prev q-tile
        projq = []  # stage-A closures of the NEXT batch

        def build_stage_a(b):
            """Allocate batch b's persistent tiles and return them with the
            list of closures that emit its projection work."""
            tb = b * S
            qT_sb = batch_pool.tile([128, S], BF16, name="qT_sb")
            # kT2 half 0: [kA; 0], half 1: [0; kB] — full-K scores keep the
            # whole PE array active so HAM stays at full clock. Zero halves
            # are initialized on the first visit of each pool buffer (b<2)
            # and inherited afterwards.
            kT2_sb = batch_pool.tile([128, 2, S], BF16, name="kT2_sb")
            vtok_sb = batch_pool.tile([128, NKT, 192], BF16, name="vtok_sb")
            xts = [
                xt00
                if (b == 0 and tt == 0)
                else xt_pool.tile([128, KT, QTILE], F32R, name="xt", tag="xt")
                for tt in range(NQT)
            ]

            issued = set()

            def prefetch(tt, b=b, xts=xts, issued=issued):
                if tt in issued or (b == 0 and tt == 0):
                    return  # only issue each tile's DMA once
                issued.add(tt)
                t0 = b * S + tt * QTILE
                nc.sync.dma_start(
                    out=xts[tt], in_=xT_r[:, :, t0 : t0 + QTILE].bitcast(F32R)
                )

            closures = []
            if b < 2:

                def init_consts(kT2_sb=kT2_sb, vtok_sb=vtok_sb):
                    # ones column + kT2 zero halves; GpSimd is otherwise idle
                    nc.gpsimd.memset(vtok_sb[:, :, 64:65], 1.0)
                    nc.gpsimd.memset(kT2_sb[64:128, 0, :], 0.0)
                    nc.gpsimd.memset(kT2_sb[0:64, 1, :], 0.0)

                closures.append(init_consts)

            for tt in range(NQT):
                cell = {}

                def proj_q(tt=tt, qT_sb=qT_sb, xts=xts, cell=cell):
                    xt = xts[tt]
                    cell["xt"] = xt
                    qP = ps.tile([128, QTILE], F32, name="qP", tag="mm", bufs=2)
                    for kt in range(KT):
                        nc.tensor.matmul(
                            qP,
                            wq_sb[:, kt, :],
                            xt[:, kt, :],
                            start=(kt == 0),
                            stop=(kt == KT - 1),
                        )
                    nc.vector.tensor_copy(
                        qT_sb[:, tt * QTILE : (tt + 1) * QTILE], qP
                    )

                def proj_k(tt=tt, kT2_sb=kT2_sb, cell=cell):
                    xt = cell["xt"]
                    kP = ps.tile([128, QTILE], F32, name="kP", tag="mm", bufs=2)
                    for kt in range(KT):
                        nc.tensor.matmul(
                            kP,
                            wk_sb[:, kt, :],
                            xt[:, kt, :],
                            start=(kt == 0),
                            stop=(kt == KT - 1),
                        )
                    nc.vector.tensor_copy(
                        kT2_sb[0:64, 0, tt * QTILE : (tt + 1) * QTILE], kP[0:64, :]
                    )
                    nc.vector.tensor_copy(
                        kT2_sb[64:128, 1, tt * QTILE : (tt + 1) * QTILE],
                        kP[64:128, :],
                    )

                def proj_v(tt=tt, prefetch=prefetch, cell=cell):
                    xt = cell["xt"]
                    vP = ps.tile([128, QTILE], F32, name="vP", tag="mm", bufs=2)
                    for kt in range(KT):
                        nc.tensor.matmul(
                            vP,
                            wv_sb[:, kt, :],
                            xt[:, kt, :],
                            start=(kt == 0),
                            stop=(kt == KT - 1),
                        )
                    vT_tmp = vtmp_pool.tile([128, QTILE], BF16, name="vT_tmp")
                    nc.vector.tensor_copy(vT_tmp, vP)
                    cell["vT"] = vT_tmp
                    if tt + 3 < NQT:
                        # xt buffer for tt is free after this group; refill
                        prefetch(tt + 3)

                def vtrans(tt=tt, vtok_sb=vtok_sb, cell=cell):
                    vT_tmp = cell["vT"]
                    for s in range(QTILE // 128):
                        vtokP = ps.tile(
                            [128, 128], BF16, name="vtokP", tag="mm", bufs=2
                        )
                        nc.tensor.transpose(
                            vtokP, vT_tmp[:, s * 128 : (s + 1) * 128], ident_sb
                        )
                        m = tt * 4 + s
                        nc.vector.tensor_copy(vtok_sb[:, m, 0:64], vtokP[:, 0:64])
                        nc.vector.tensor_copy(
                            vtok_sb[:, m, 128:192], vtokP[:, 64:128]
                        )

                closures += [proj_q, proj_k, proj_v, vtrans]
            return (qT_sb, kT2_sb, vtok_sb), prefetch, closures

        def attention(b, tiles):
            qT_sb, kT2_sb, vtok_sb = tiles
            tb = b * S
            for qi in range(NQT):
                q0 = qi * QTILE  # batch-local q base
                nk = 4 * qi + 4  # k-tiles for this q-tile (block-causal)
                ctxAB = ps.tile(
                    [128, 2 * QTILE], F32, name="ctxAB", tag="ctx", bufs=1
                )

                def geom(m, qi=qi):
                    d_off = m - 4 * qi
                    if d_off >= 0:
                        return QTILE - 128 * d_off, 128 * d_off, True
                    return QTILE, 0, False

                exps = {}
                for i in range(nk + LAG):
                    # Staggered pops: finish_rb (prev q-tile) at i==2, once
                    # its s2 copies are in; finish_oproj at i==5, once ctxn
                    # is normalized; next batch's projections fill the other
                    # odd slots so the PE outpaces the scalar exp stream.
                    if i == 2 and finishq:
                        finishq.pop(0)()
                    elif i == 5 and finishq:
                        finishq.pop(0)()
                    elif projq and i % 2 == 1:
                        projq.pop(0)()
                    if i < nk:
                        m = i
                        width, qoff, diag = geom(m)
                        sAB = ps.tile(
                            [128, 2 * QTILE], F32, name="sAB", tag="sc", bufs=2
                        )
                        nc.tensor.matmul(
                            sAB[:, 0:width],
                            kT2_sb[:, 0, m * 128 : (m + 1) * 128],
                            qT_sb[:, q0 + qoff : q0 + QTILE],
                            start=True,
                            stop=True,
                        )
                        nc.tensor.matmul(
                            sAB[:, QTILE : QTILE + width],
                            kT2_sb[:, 1, m * 128 : (m + 1) * 128],
                            qT_sb[:, q0 + qoff : q0 + QTILE],
                            start=True,
                            stop=True,
                        )
                        # one wide exp covers both heads; the gap region
                        # [width:QTILE] holds unread junk for diag tiles.
                        eAB = exp_pool.tile(
                            [128, 2 * QTILE], BF16, name="eAB", tag="exp"
                        )
                        nc.scalar.activation(
                            eAB[:, 0 : QTILE + width],
                            sAB[:, 0 : QTILE + width],
                            EXP_FUNC,
                            scale=INV_SQRT_DK,
                        )
                        if diag:
                            nc.vector.tensor_mul(eAB[:, 0:128], eAB[:, 0:128], tri_sb)
                            nc.vector.tensor_mul(
                                eAB[:, QTILE : QTILE + 128],
                                eAB[:, QTILE : QTILE + 128],
                                tri_sb,
                            )
                        exps[m] = eAB

                    j = i - LAG
                    if j >= 0:
                        width, qoff, _ = geom(j)
                        first = j == 0
                        last = j == nk - 1
                        eAB = exps.pop(j)
                        nc.tensor.matmul(
                            ctxAB[:, qoff:QTILE],
                            vtok_sb[:, j, 0:128],
                            eAB[:, 0:width],
                            start=first,
                            stop=last,
                            skip_group_check=True,
                        )
                        nc.tensor.matmul(
                            ctxAB[:, QTILE + qoff : 2 * QTILE],
                            vtok_sb[:, j, 64:192],
                            eAB[:, QTILE : QTILE + width],
                            start=first,
                            stop=last,
                            skip_group_check=True,
                        )

                # normalization part 1 (immediate, frees the ctx PSUM slots):
                # sumB is already on partition 0 of the B half (shared ones
                # col) and sumA on partition 64 of the A half — both are
                # same-partition scalar copies into s2; rows 1:63 of s2 hold
                # zeros (memset once per pool buffer) so the K=65 broadcast
                # matmul ignores the junk. ctx copies go on the vector
                # engine so the scalar EXP stream is not delayed.
                s2 = small_pool.tile([65, 2, QTILE], F32R, name="s2")
                if b == 0 and qi < 2:
                    nc.gpsimd.memset(s2[:, 0, :].bitcast(F32), 0.0)
                nc.vector.tensor_copy(s2[0:1, 0, :], ctxAB[0:1, QTILE : 2 * QTILE])
                nc.vector.tensor_copy(s2[64:65, 0, :], ctxAB[64:65, 0:QTILE])
                ctxn = ctxn_pool.tile([128, QTILE], F32R, name="ctxn")
                nc.vector.tensor_copy(ctxn[0:64, :], ctxAB[0:64, 0:QTILE])
                nc.vector.tensor_copy(
                    ctxn[64:128, :], ctxAB[64:128, QTILE : 2 * QTILE]
                )

                def finish_rb(qi=qi, ctxn=ctxn, s2=s2):
                    # deferred stage 1: K=65 indicator matmul broadcasts both
                    # sums to [128, QTILE] PSUM (row 0 = sumB -> partitions
                    # 64:128, row 64 = sumA -> 0:64); fast-approx reciprocal
                    # + normalize.
                    rbP = ps.tile([128, QTILE], F32, name="rbP", tag="mm", bufs=2)
                    nc.tensor.matmul(
                        rbP, ind_sb, s2[0:65, 0, :], start=True, stop=True
                    )
                    rb_sb = small_pool.tile([128, QTILE], F32, name="rb_sb")
                    nc.vector.reciprocal_approx_fast(out=rb_sb, in_=rbP)
                    nc.vector.tensor_mul(ctxn, ctxn, rb_sb.bitcast(F32R))

                is_last = b == B - 1 and qi == NQT - 1

                def finish_oproj(qi=qi, q0=q0, tb=tb, ctxn=ctxn, split=is_last):
                    # deferred stage 2: output projection (ctxn is normalized
                    # by the time this runs several m-iterations later). Only
                    # the kernel's very last writeback is split in two (DMA
                    # overlaps the remaining matmuls -> shorter tail); the
                    # others stay single-DMA to keep sync-queue issues low.
                    o_all = oall_pool.tile([128, KT, QTILE], F32, name="o_all")
                    odst = outT_r[:, :, tb + q0 : tb + q0 + QTILE]
                    engs = [nc.scalar.copy, nc.vector.tensor_copy]
                    for jt in range(KT):
                        oP = ps.tile([128, QTILE], F32, name="oP", tag="mm", bufs=2)
                        nc.tensor.matmul(
                            oP, wo_sb[:, jt, :], ctxn, start=True, stop=True
                        )
                        engs[jt % 3](o_all[:, jt, :], oP)
                        if split and jt == KT // 2 - 1:
                            nc.sync.dma_start(
                                out=odst[:, 0 : KT // 2, :],
                                in_=o_all[:, 0 : KT // 2, :],
                            )
                    if split:
                        nc.sync.dma_start(
                            out=odst[:, KT // 2 :, :], in_=o_all[:, KT // 2 :, :]
                        )
                    else:
                        nc.sync.dma_start(out=odst, in_=o_all)

                finishq.extend([finish_rb, finish_oproj])

        # batch 0's stage A runs inline (nothing to overlap with); its xt
        # prefetches ride the pool rotation as the closures execute.
        tiles0, prefetch0, cl0 = build_stage_a(0)
        for tt in range(1, NQT):
            prefetch0(tt)
        for c in cl0:
            c()
        tiles = tiles0
        for b in range(B):
            if b + 1 < B:
                next_tiles, next_prefetch, next_cl = build_stage_a(b + 1)
                # keep the xt pipeline 3 tiles deep across the batch seam
                for tt in range(min(3, NQT)):
                    next_prefetch(tt)
                projq.extend(next_cl)
            attention(b, tiles)
            while projq:
                projq.pop(0)()
            if b + 1 < B:
                tiles = next_tiles

        while finishq:
            finishq.pop(0)()

    nc.compile()
    return nc


_NC = None


def _get_nc():
    global _NC
    if _NC is None:
        _NC = build_nc()
    return _NC


def make_in_maps(x, W_Q, W_K, W_V, W_O):
    xTh = np.ascontiguousarray(np.asarray(x, dtype=np.float32).reshape(T, D).T)
    W_Q = np.asarray(W_Q, dtype=np.float32)
    W_K = np.asarray(W_K, dtype=np.float32)
    W_V = np.asarray(W_V, dtype=np.float32)
    W_O = np.asarray(W_O, dtype=np.float32)
    import ml_dtypes

    tri = np.triu(np.ones((128, 128), dtype=ml_dtypes.bfloat16))  # tri[k,q]=1 iff q>=k
    ind2 = np.zeros((65, 128), dtype=np.float32)
    ind2[0, 64:128] = 1.0  # row 0 = sumB -> partitions 64:128
    ind2[64, 0:64] = 1.0  # row 64 = sumA -> partitions 0:64
    ident = np.eye(128, dtype=ml_dtypes.bfloat16)
    in_maps = []
    for c in range(NCORES):
        sl = slice(c * 128, (c + 1) * 128)
        in_maps.append(
            {
                "xT": xTh,
                "wq": np.ascontiguousarray(W_Q[sl, :].T),
                "wk": np.ascontiguousarray(W_K[sl, :].T),
                "wv": np.ascontiguousarray(W_V[sl, :].T),
                "wo": np.ascontiguousarray(W_O.T[sl, :]),
                "tri": tri,
                "ind": ind2,
                "ident": ident,
            }
        )
    return in_maps


def kernel(x, W_Q, W_K, W_V, W_O, _results_hook=None):
    nc = _get_nc()
    in_maps = make_in_maps(x, W_Q, W_K, W_V, W_O)
    res = run_bass_kernel_spmd(nc, in_maps, list(range(NCORES)))
    if _results_hook is not None:
        _results_hook(res)
    acc = np.zeros((D, T), dtype=np.float64)
    for c in range(NCORES):
        acc += res.results[c]["outT"]
    out = np.ascontiguousarray(acc.T).reshape(B, S, D).astype(np.float32)
    return out


# revision 37
# speedup vs baseline: 1.0289x; 1.0289x over previous
"""Causal multi-head self-attention on 8 TRN2 NeuronCores.

Sharding: tensor-parallel over heads. 16 heads / 8 cores = 2 heads per core.
Each core computes q/k/v projections for its 2 heads (feature-major via
fp32r matmuls), block-causal attention (scores kept k-major so softmax sums
come from a fused ones-column in the attn@v matmul and no transposes are
needed), and a partial output projection against its 128-column slice of
W_O. The host sums the 8 partial outputs.

Layouts on core c (heads 2c, 2c+1 = "A", "B"):
  qT/kT  [128, 2048]  feature-major; rows 0:64 head A dk, 64:128 head B
  vtok   [128, 16, 192] token-major v (PE-transposed): cols 0:64 vA, col 64
         a shared ones column, cols 128:192 vB. Head A's ctx lhsT slice is
         cols 0:128 (ctxA on partitions 0:64, sumA on 64); head B's slice is
         cols 64:192, so the same ones column becomes its col 0 (sumB on
         partition 0) and vB lands on partitions 64:128 — no partition-shift
         DMA is needed to assemble ctxn.
  scoresT[128 k-tok, <=512 q-tok] per (q-tile, k-tile); exp'd on ScalarE
  ctxAB  [128, 2*512] PSUM; A half rows 0:64 + sum row 64, B half sum row 0
         + rows 64:128
  out    partial [1024, 8192] feature-major; host sums over cores + transposes

Pipelining: the whole kernel is one deferred-work machine. Attention of
batch b interleaves (as pop-closures in its k-tile loop) the q/k/v
projections + v-transposes of batch b+1, plus the softmax-normalization
(finish_rb) and output-projection (finish_oproj) of the previous q-tile.
This keeps the PE stream dense (pstate stays at max clock) and gives the
scalar engine's exp stream (~1us per k-iter, the attention pacer) slack.

Softmax denominators: sumA sits on partition 64, sumB on partition 0 of the
two ctx halves; both are same-partition scalar copies into s2, whose rows
1:63 hold persistent zeros, and a K=65 indicator matmul broadcasts them to
[128, QTILE]. reciprocal_approx_fast (~18 bits) + one DVE multiply
normalize ctx.

Toolchain constraints honored here: col-offset tile_position is illegal for
4-byte matmul dtypes; fp32r consumers need fp32r-typed producers; x is
transposed on the host so every DMA has a contiguous innermost run (>=2KB);
fp32r matmuls with moving dim >= 256 run at full PE rate.
"""

import numpy as np
from contextlib import ExitStack

import concourse.bass as bass
import concourse.tile as tile
from concourse import bacc, mybir
from concourse.bass_utils import run_bass_kernel_spmd

F32 = mybir.dt.float32
F32R = mybir.dt.float32r
BF16 = mybir.dt.bfloat16

B, S, D, H = 4, 2048, 1024, 16
DK = D // H  # 64
NCORES = 8
T = B * S  # 8192 tokens
KT = D // 128  # 8 contraction tiles for projections
QTILE = 512  # q-tile width (tokens)
KTILE = 128  # k-tile width (tokens)
NQT = S // QTILE  # 4 q-tiles per batch
NKT = S // KTILE  # 16 k-tiles per batch
LAG = 2  # ctx matmuls trail scores by this many k-tiles
EXP_FUNC = mybir.ActivationFunctionType.Exp
INV_SQRT_DK = 1.0 / np.sqrt(DK)


def build_nc():
    nc = bacc.Bacc("TRN2", target_bir_lowering=False, debug=False)

    xT = nc.dram_tensor("xT", [D, T], F32, kind="ExternalInput").ap()
    wq = nc.dram_tensor("wq", [D, 128], F32, kind="ExternalInput").ap()
    wk = nc.dram_tensor("wk", [D, 128], F32, kind="ExternalInput").ap()
    wv = nc.dram_tensor("wv", [D, 128], F32, kind="ExternalInput").ap()
    wo = nc.dram_tensor("wo", [128, D], F32, kind="ExternalInput").ap()
    tri = nc.dram_tensor("tri", [128, 128], BF16, kind="ExternalInput").ap()
    ind = nc.dram_tensor("ind", [65, 128], F32, kind="ExternalInput").ap()
    ident = nc.dram_tensor("ident", [128, 128], BF16, kind="ExternalInput").ap()
    outT = nc.dram_tensor("outT", [D, T], F32, kind="ExternalOutput").ap()

    xT_r = xT.rearrange("(kt p) t -> p kt t", p=128)
    outT_r = outT.rearrange("(jt p) t -> p jt t", p=128)

    with ExitStack() as ctx:
        tc = ctx.enter_context(tile.TileContext(nc))
        consts = ctx.enter_context(tc.tile_pool(name="consts", bufs=1))
        xt_pool = ctx.enter_context(tc.tile_pool(name="xt_pool", bufs=3))
        batch_pool = ctx.enter_context(tc.tile_pool(name="batch_pool", bufs=2))
        vtmp_pool = ctx.enter_context(tc.tile_pool(name="vtmp_pool", bufs=3))
        exp_pool = ctx.enter_context(tc.tile_pool(name="exp_pool", bufs=4))
        ctxn_pool = ctx.enter_context(tc.tile_pool(name="ctxn_pool", bufs=2))
        oall_pool = ctx.enter_context(tc.tile_pool(name="oall_pool", bufs=1))
        small_pool = ctx.enter_context(tc.tile_pool(name="small_pool", bufs=2))
        ps = ctx.enter_context(tc.tile_pool(name="ps", bufs=1, space="PSUM"))

        # --- constants / weights (persistent) ---
        # Ordered so the head of the kernel only waits for wq + the first
        # x chunk: wo (first needed ~40us in) and small consts go later.
        wq_sb = consts.tile([128, KT, 128], F32R)
        wq_src = wq.rearrange("(kt p) m -> p kt m", p=128).bitcast(F32R)
        nc.sync.dma_start(out=wq_sb[:, 0 : KT // 2, :], in_=wq_src[:, 0 : KT // 2, :])
        nc.sync.dma_start(out=wq_sb[:, KT // 2 :, :], in_=wq_src[:, KT // 2 :, :])
        # first x tile, split per-kt and issued directly after wq so the
        # first projection matmul starts after ~768KB instead of ~4MB
        xt00 = xt_pool.tile([128, KT, QTILE], F32R, name="xt", tag="xt")
        for kt in range(KT):
            nc.sync.dma_start(
                out=xt00[:, kt, :], in_=xT_r[:, kt, 0:QTILE].bitcast(F32R)
            )
        wk_sb = consts.tile([128, KT, 128], F32R)
        nc.sync.dma_start(
            out=wk_sb, in_=wk.rearrange("(kt p) m -> p kt m", p=128).bitcast(F32R)
        )
        wv_sb = consts.tile([128, KT, 128], F32R)
        nc.sync.dma_start(
            out=wv_sb, in_=wv.rearrange("(kt p) m -> p kt m", p=128).bitcast(F32R)
        )
        tri_sb = consts.tile([128, 128], BF16)
        nc.sync.dma_start(out=tri_sb, in_=tri)
        ind_sb = consts.tile([65, 128], F32R)
        nc.sync.dma_start(out=ind_sb, in_=ind.bitcast(F32R))
        ident_sb = consts.tile([128, 128], BF16)
        nc.sync.dma_start(out=ident_sb, in_=ident)
        wo_sb = consts.tile([128, KT, 128], F32R)
        nc.sync.dma_start(
            out=wo_sb, in_=wo.rearrange("p (jt m) -> p jt m", jt=KT).bitcast(F32R)
        )

        finishq = []  # [finish_rb(qi), finish_oproj(qi)] of the prev q-tile
        projq = []  # stage-A closures of the NEXT batch

        def build_stage_a(b):
            """Allocate batch b's persistent tiles and return them with the
            list of closures that emit its projection work."""
            tb = b * S
            qT_sb = batch_pool.tile([128, S], BF16, name="qT_sb")
            # kT2 half 0: [kA; 0], half 1: [0; kB] — full-K scores keep the
            # whole PE array active so HAM stays at full clock. Zero halves
            # are initialized on the first visit of each pool buffer (b<2)
            # and inherited afterwards.
            kT2_sb = batch_pool.tile([128, 2, S], BF16, name="kT2_sb")
            vtok_sb = batch_pool.tile([128, NKT, 192], BF16, name="vtok_sb")
            xts = [
                xt00
                if (b == 0 and tt == 0)
                else xt_pool.tile([128, KT, QTILE], F32R, name="xt", tag="xt")
                for tt in range(NQT)
            ]

            issued = set()

            def prefetch(tt, b=b, xts=xts, issued=issued):
                if tt in issued or (b == 0 and tt == 0):
                    return  # only issue each tile's DMA once
                issued.add(tt)
                t0 = b * S + tt * QTILE
                nc.sync.dma_start(
                    out=xts[tt], in_=xT_r[:, :, t0 : t0 + QTILE].bitcast(F32R)
                )

            closures = []
            if b < 2:

                def init_consts(kT2_sb=kT2_sb, vtok_sb=vtok_sb):
                    # ones column + kT2 zero halves; GpSimd is otherwise idle
                    nc.gpsimd.memset(vtok_sb[:, :, 64:65], 1.0)
                    nc.gpsimd.memset(kT2_sb[64:128, 0, :], 0.0)
                    nc.gpsimd.memset(kT2_sb[0:64, 1, :], 0.0)

                closures.append(init_consts)

            for tt in range(NQT):
                cell = {}

                def proj_q(tt=tt, qT_sb=qT_sb, xts=xts, cell=cell):
                    xt = xts[tt]
                    cell["xt"] = xt
                    qP = ps.tile([128, QTILE], F32, name="qP", tag="mm", bufs=3)
                    for kt in range(KT):
                        nc.tensor.matmul(
                            qP,
                            wq_sb[:, kt, :],
                            xt[:, kt, :],
                            start=(kt == 0),
                            stop=(kt == KT - 1),
                        )
                    nc.vector.tensor_copy(
                        qT_sb[:, tt * QTILE : (tt + 1) * QTILE], qP
                    )

                def proj_k(tt=tt, kT2_sb=kT2_sb, cell=cell):
                    xt = cell["xt"]
                    kP = ps.tile([128, QTILE], F32, name="kP", tag="mm", bufs=3)
                    for kt in range(KT):
                        nc.tensor.matmul(
                            kP,
                            wk_sb[:, kt, :],
                            xt[:, kt, :],
                            start=(kt == 0),
                            stop=(kt == KT - 1),
                        )
                    nc.vector.tensor_copy(
                        kT2_sb[0:64, 0, tt * QTILE : (tt + 1) * QTILE], kP[0:64, :]
                    )
                    nc.vector.tensor_copy(
                        kT2_sb[64:128, 1, tt * QTILE : (tt + 1) * QTILE],
                        kP[64:128, :],
                    )

                def proj_v(tt=tt, prefetch=prefetch, cell=cell):
                    xt = cell["xt"]
                    vP = ps.tile([128, QTILE], F32, name="vP", tag="mm", bufs=3)
                    for kt in range(KT):
                        nc.tensor.matmul(
                            vP,
                            wv_sb[:, kt, :],
                            xt[:, kt, :],
                            start=(kt == 0),
                            stop=(kt == KT - 1),
                        )
                    vT_tmp = vtmp_pool.tile([128, QTILE], BF16, name="vT_tmp")
                    nc.vector.tensor_copy(vT_tmp, vP)
                    cell["vT"] = vT_tmp
                    if tt + 3 < NQT:
                        # xt buffer for tt is free after this group; refill
                        prefetch(tt + 3)

                def vtrans(tt=tt, vtok_sb=vtok_sb, cell=cell):
                    vT_tmp = cell["vT"]
                    for s in range(QTILE // 128):
                        vtokP = ps.tile(
                            [128, 128], BF16, name="vtokP", tag="mm", bufs=3
                        )
                        nc.tensor.transpose(
                            vtokP, vT_tmp[:, s * 128 : (s + 1) * 128], ident_sb
                        )
                        m = tt * 4 + s
                        nc.vector.tensor_copy(vtok_sb[:, m, 0:64], vtokP[:, 0:64])
                        nc.vector.tensor_copy(
                            vtok_sb[:, m, 128:192], vtokP[:, 64:128]
                        )

                closures += [proj_q, proj_k, proj_v, vtrans]
            return (qT_sb, kT2_sb, vtok_sb), prefetch, closures

        def attention(b, tiles):
            qT_sb, kT2_sb, vtok_sb = tiles
            tb = b * S
            for qi in range(NQT):
                q0 = qi * QTILE  # batch-local q base
                nk = 4 * qi + 4  # k-tiles for this q-tile (block-causal)
                ctxAB = ps.tile(
                    [128, 2 * QTILE], F32, name="ctxAB", tag="ctx", bufs=1
                )

                def geom(m, qi=qi):
                    d_off = m - 4 * qi
                    if d_off >= 0:
                        return QTILE - 128 * d_off, 128 * d_off, True
                    return QTILE, 0, False

                exps = {}
                for i in range(nk + LAG):
                    # Staggered pops: finish_rb (prev q-tile) at i==2, once
                    # its s2 copies are in; finish_oproj at i==5, once ctxn
                    # is normalized; next batch's projections fill the other
                    # odd slots so the PE outpaces the scalar exp stream.
                    if i == 2 and finishq:
                        finishq.pop(0)()
                    elif i == 5 and finishq:
                        finishq.pop(0)()
                    elif projq and i % 2 == 1:
                        projq.pop(0)()
                    if i < nk:
                        m = i
                        width, qoff, diag = geom(m)
                        sAB = ps.tile(
                            [128, 2 * QTILE], BF16, name="sAB", tag="sc", bufs=3
                        )
                        nc.tensor.matmul(
                            sAB[:, 0:width],
                            kT2_sb[:, 0, m * 128 : (m + 1) * 128],
                            qT_sb[:, q0 + qoff : q0 + QTILE],
                            start=True,
                            stop=True,
                        )
                        nc.tensor.matmul(
                            sAB[:, QTILE : QTILE + width],
                            kT2_sb[:, 1, m * 128 : (m + 1) * 128],
                            qT_sb[:, q0 + qoff : q0 + QTILE],
                            start=True,
                            stop=True,
                        )
                        # one wide exp covers both heads; the gap region
                        # [width:QTILE] holds unread junk for diag tiles.
                        eAB = exp_pool.tile(
                            [128, 2 * QTILE], BF16, name="eAB", tag="exp"
                        )
                        nc.scalar.activation(
                            eAB[:, 0 : QTILE + width],
                            sAB[:, 0 : QTILE + width],
                            EXP_FUNC,
                            scale=INV_SQRT_DK,
                        )
                        if diag:
                            nc.vector.tensor_mul(eAB[:, 0:128], eAB[:, 0:128], tri_sb)
                            nc.vector.tensor_mul(
                                eAB[:, QTILE : QTILE + 128],
                                eAB[:, QTILE : QTILE + 128],
                                tri_sb,
                            )
                        exps[m] = eAB

                    j = i - LAG
                    if j >= 0:
                        width, qoff, _ = geom(j)
                        first = j == 0
                        last = j == nk - 1
                        eAB = exps.pop(j)
                        nc.tensor.matmul(
                            ctxAB[:, qoff:QTILE],
                            vtok_sb[:, j, 0:128],
                            eAB[:, 0:width],
                            start=first,
                            stop=last,
                            skip_group_check=True,
                        )
                        nc.tensor.matmul(
                            ctxAB[:, QTILE + qoff : 2 * QTILE],
                            vtok_sb[:, j, 64:192],
                            eAB[:, QTILE : QTILE + width],
                            start=first,
                            stop=last,
                            skip_group_check=True,
                        )

                # normalization part 1 (immediate, frees the ctx PSUM slots):
                # sumB is already on partition 0 of the B half (shared ones
                # col) and sumA on partition 64 of the A half — both are
                # same-partition scalar copies into s2; rows 1:63 of s2 hold
                # zeros (memset once per pool buffer) so the K=65 broadcast
                # matmul ignores the junk. ctx copies go on the vector
                # engine so the scalar EXP stream is not delayed.
                s2 = small_pool.tile([65, 2, QTILE], F32R, name="s2")
                if b == 0 and qi < 2:
                    nc.gpsimd.memset(s2[:, 0, :].bitcast(F32), 0.0)
                nc.vector.tensor_copy(s2[0:1, 0, :], ctxAB[0:1, QTILE : 2 * QTILE])
                nc.vector.tensor_copy(s2[64:65, 0, :], ctxAB[64:65, 0:QTILE])
                ctxn = ctxn_pool.tile([128, QTILE], F32R, name="ctxn")
                nc.vector.tensor_copy(ctxn[0:64, :], ctxAB[0:64, 0:QTILE])
                nc.vector.tensor_copy(
                    ctxn[64:128, :], ctxAB[64:128, QTILE : 2 * QTILE]
                )

                def finish_rb(qi=qi, ctxn=ctxn, s2=s2):
                    # deferred stage 1: K=65 indicator matmul broadcasts both
                    # sums to [128, QTILE] PSUM (row 0 = sumB -> partitions
                    # 64:128, row 64 = sumA -> 0:64); fast-approx reciprocal
                    # + normalize.
                    rbP = ps.tile([128, QTILE], F32, name="rbP", tag="mm", bufs=3)
                    nc.tensor.matmul(
                        rbP, ind_sb, s2[0:65, 0, :], start=True, stop=True
                    )
                    rb_sb = small_pool.tile([128, QTILE], F32, name="rb_sb")
                    nc.vector.reciprocal_approx_fast(out=rb_sb, in_=rbP)
                    nc.vector.tensor_mul(ctxn, ctxn, rb_sb.bitcast(F32R))

                is_last = b == B - 1 and qi == NQT - 1

                def finish_oproj(qi=qi, q0=q0, tb=tb, ctxn=ctxn, split=is_last):
                    # deferred stage 2: output projection (ctxn is normalized
                    # by the time this runs several m-iterations later). Only
                    # the kernel's very last writeback is split in two (DMA
                    # overlaps the remaining matmuls -> shorter tail); the
                    # others stay single-DMA to keep sync-queue issues low.
                    o_all = oall_pool.tile([128, KT, QTILE], F32, name="o_all")
                    odst = outT_r[:, :, tb + q0 : tb + q0 + QTILE]
                    engs = [nc.scalar.copy, nc.vector.tensor_copy]
                    for jt in range(KT):
                        oP = ps.tile([128, QTILE], F32, name="oP", tag="mm", bufs=3)
                        nc.tensor.matmul(
                            oP, wo_sb[:, jt, :], ctxn, start=True, stop=True
                        )
                        engs[jt % 3](o_all[:, jt, :], oP)
                        if split and jt == KT // 2 - 1:
                            nc.sync.dma_start(
                                out=odst[:, 0 : KT // 2, :],
                                in_=o_all[:, 0 : KT // 2, :],
                            )
                    if split:
                        nc.sync.dma_start(
                            out=odst[:, KT // 2 :, :], in_=o_all[:, KT // 2 :, :]
                        )
                    else:
                        nc.sync.dma_start(out=odst, in_=o_all)

                finishq.extend([finish_rb, finish_oproj])

        # batch 0's stage A runs inline (nothing to overlap with); its xt
        # prefetches ride the pool rotation as the closures execute.
        tiles0, prefetch0, cl0 = build_stage_a(0)
        for tt in range(1, NQT):
            prefetch0(tt)
        for c in cl0:
            c()
        tiles = tiles0
        for b in range(B):
            if b + 1 < B:
                next_tiles, next_prefetch, next_cl = build_stage_a(b + 1)
                # keep the xt pipeline 3 tiles deep across the batch seam
                for tt in range(min(3, NQT)):
                    next_prefetch(tt)
                projq.extend(next_cl)
            attention(b, tiles)
            while projq:
                projq.pop(0)()
            if b + 1 < B:
                tiles = next_tiles

        while finishq:
            finishq.pop(0)()

    nc.compile()
    return nc


_NC = None


def _get_nc():
    global _NC
    if _NC is None:
        _NC = build_nc()
    return _NC


def make_in_maps(x, W_Q, W_K, W_V, W_O):
    xTh = np.ascontiguousarray(np.asarray(x, dtype=np.float32).reshape(T, D).T)
    W_Q = np.asarray(W_Q, dtype=np.float32)
    W_K = np.asarray(W_K, dtype=np.float32)
    W_V = np.asarray(W_V, dtype=np.float32)
    W_O = np.asarray(W_O, dtype=np.float32)
    import ml_dtypes

    tri = np.triu(np.ones((128, 128), dtype=ml_dtypes.bfloat16))  # tri[k,q]=1 iff q>=k
    ind2 = np.zeros((65, 128), dtype=np.float32)
    ind2[0, 64:128] = 1.0  # row 0 = sumB -> partitions 64:128
    ind2[64, 0:64] = 1.0  # row 64 = sumA -> partitions 0:64
    ident = np.eye(128, dtype=ml_dtypes.bfloat16)
    in_maps = []
    for c in range(NCORES):
        sl = slice(c * 128, (c + 1) * 128)
        in_maps.append(
            {
                "xT": xTh,
                "wq": np.ascontiguousarray(W_Q[sl, :].T),
                "wk": np.ascontiguousarray(W_K[sl, :].T),
                "wv": np.ascontiguousarray(W_V[sl, :].T),
                "wo": np.ascontiguousarray(W_O.T[sl, :]),
                "tri": tri,
                "ind": ind2,
                "ident": ident,
            }
        )
    return in_maps


def kernel(x, W_Q, W_K, W_V, W_O, _results_hook=None):
    nc = _get_nc()
    in_maps = make_in_maps(x, W_Q, W_K, W_V, W_O)
    res = run_bass_kernel_spmd(nc, in_maps, list(range(NCORES)))
    if _results_hook is not None:
        _results_hook(res)
    acc = np.zeros((D, T), dtype=np.float64)
    for c in range(NCORES):
        acc += res.results[c]["outT"]
    out = np.ascontiguousarray(acc.T).reshape(B, S, D).astype(np.float32)
    return out
